# revision 3
# baseline (speedup 1.0000x reference)
"""Causal attention (B=4, S=4096, D=64, fp32) on 8 Trainium2 NeuronCores.

Sharding: core = 2*b + s handles batch b (4 batches x 2 cores). Within a
batch, the 4096 q rows form 8 chunks of 512; chunk c needs k-tiles
0..4c+3 (causal). Core s=0 takes chunks {1,3,5,7} (k-tile counts
{8,16,24,32}), core s=1 takes chunks {0,2,4,6} (counts {4,12,20,28})
padded up to the same {8,16,24,32} so all 8 cores run one identical SPMD
program; pad k-tiles are masked via a 65th contraction row (-8192 bias ->
exp underflows to exactly 0).

Layout: scores are computed transposed, S^T[k,q] = K Q^T, with the
contraction dim d on SBUF partitions, so softmax normalization can be
deferred (a ones-column appended to V accumulates the row sums during the
P^T V matmul) and P^T feeds the PV matmul with no transposes. Diagonal
128x128 triangles are zeroed with affine_select at fixed program
positions (tail k-tiles are fed from per-chunk "slab" inputs that the
host orders as [full/pad x4, diag x4]).
"""

import numpy as np

import concourse.bass as bass  # noqa: F401  (keeps engine classes registered)
import concourse.mybir as mybir
from concourse import bacc
from concourse.tile import TileContext
from concourse.masks import make_identity
from concourse.bass_utils import run_bass_kernel_spmd

B, S, D = 4, 4096, 64
NCORES = 8
SLOT_C = (8, 16, 24, 32)  # k-tiles per chunk slot (uniform across cores)
NEG = -8192.0
F32 = mybir.dt.float32
F32R = mybir.dt.float32r

_prog_cache = None


def _build_program():
    global _prog_cache
    if _prog_cache is not None:
        return _prog_cache

    nc = bacc.Bacc("TRN2", target_bir_lowering=False, debug=False)
    qt_d = nc.declare_dram_parameter("qt", [65, 2048], F32R, isOutput=False)
    ktm_d = nc.declare_dram_parameter("ktm", [64, 3072], F32R, isOutput=False)
    kts_d = nc.declare_dram_parameter("kts", [65, 4096], F32R, isOutput=False)
    vm_d = nc.declare_dram_parameter("vm", [128, 1560], F32R, isOutput=False)
    vs_d = nc.declare_dram_parameter("vs", [128, 2080], F32R, isOutput=False)
    o_d = nc.declare_dram_parameter("o", [2048, 64], F32, isOutput=True)
    EXP = mybir.ActivationFunctionType.Exp

    with TileContext(nc) as tc:
        with (
            tc.tile_pool(name="cons", bufs=1) as cons,
            tc.tile_pool(name="data", bufs=1) as data,
            tc.tile_pool(name="pp", bufs=3) as pp,
            tc.tile_pool(name="ep", bufs=2) as ep,
            tc.tile_pool(name="ps_sc", bufs=2, space="PSUM") as ps_sc,
            tc.tile_pool(name="ps_acc", bufs=2, space="PSUM") as ps_acc,
            tc.tile_pool(name="ps_t", bufs=2, space="PSUM") as ps_t,
        ):
            ident = cons.tile([128, 128], F32)
            make_identity(nc, ident[:])

            qt = data.tile([65, 2048], F32R)
            nc.sync.dma_start(out=qt[:], in_=qt_d[:])
            kts = data.tile([65, 4096], F32R)
            vs = data.tile([128, 2080], F32R)
            for m in range(4):
                nc.sync.dma_start(
                    out=kts[:, 1024 * m : 1024 * (m + 1)],
                    in_=kts_d[:, 1024 * m : 1024 * (m + 1)],
                )
                nc.sync.dma_start(
                    out=vs[:, 520 * m : 520 * (m + 1)],
                    in_=vs_d[:, 520 * m : 520 * (m + 1)],
                )
            ktm = data.tile([64, 3072], F32R)
            vm = data.tile([128, 1560], F32R)
            for g2 in range(4):
                nc.sync.dma_start(
                    out=ktm[:, 768 * g2 : 768 * (g2 + 1)],
                    in_=ktm_d[:, 768 * g2 : 768 * (g2 + 1)],
                )
                nc.sync.dma_start(
                    out=vm[:, 390 * g2 : 390 * (g2 + 1)],
                    in_=vm_d[:, 390 * g2 : 390 * (g2 + 1)],
                )

            for m in range(4):
                C = SLOT_C[m]
                q_sl = slice(512 * m, 512 * (m + 1))
                acc = ps_acc.tile([65, 512], F32, tag="acc")

                def emit_pv(pt, t0, C=C, m=m, acc=acc):
                    for d_ in range(2):
                        t = t0 + d_
                        ptile = pt[:, 512 * d_ : 512 * (d_ + 1)]
                        if t >= C - 4:
                            g = t - (C - 4)
                            nc.gpsimd.affine_select(
                                out=ptile,
                                in_=ptile,
                                compare_op=mybir.AluOpType.is_ge,
                                fill=0.0,
                                base=-128 * g,
                                pattern=[[1, 512]],
                                channel_multiplier=-1,
                            )
                        if t <= C - 9:
                            vt = vm[:, 65 * t : 65 * (t + 1)]
                        else:
                            p = t - (C - 8)
                            vt = vs[:, 520 * m + 65 * p : 520 * m + 65 * (p + 1)]
                        nc.tensor.matmul(
                            acc[:],
                            vt,
                            ptile,
                            start=(t == 0),
                            stop=(t == C - 1),
                        )

                pending = None
                for t0 in range(0, C, 2):
                    sc = ps_sc.tile([128, 1024], F32, tag="sc")
                    for d_ in range(2):
                        t = t0 + d_
                        if t <= C - 9:
                            lhsT = ktm[:, 128 * t : 128 * (t + 1)]
                            rhs = qt[0:64, q_sl]
                        else:
                            p = t - (C - 8)
                            lhsT = kts[
                                :, 1024 * m + 128 * p : 1024 * m + 128 * (p + 1)
                            ]
                            rhs = qt[0:65, q_sl]
                        nc.tensor.matmul(
                            sc[:, 512 * d_ : 512 * (d_ + 1)],
                            lhsT,
                            rhs,
                            start=True,
                            stop=True,
                        )
                    pt = pp.tile([128, 1024], F32R, tag="pt")
                    nc.scalar.activation(pt[:], sc[:], EXP, scale=0.125)
                    if pending is not None:
                        emit_pv(*pending)
                    pending = (pt, t0)
                emit_pv(*pending)

                osb = ep.tile([65, 512], F32, tag="osb")
                nc.vector.tensor_copy(osb[:], acc[:])
                oo = ep.tile([128, 256], F32, tag="oo")
                for j in range(4):
                    tp = ps_t.tile([128, 65], F32, tag="tp")
                    nc.tensor.transpose(
                        tp[:], osb[:, 128 * j : 128 * (j + 1)], ident[0:65, 0:65]
                    )
                    rec = ep.tile([128, 1], F32, tag="rec", bufs=8)
                    nc.vector.reciprocal(rec[:], tp[:, 64:65])
                    nc.vector.tensor_scalar_mul(
                        oo[:, 64 * j : 64 * (j + 1)], tp[:, 0:64], rec[:]
                    )
                nc.sync.dma_start(
                    out=o_d[512 * m : 512 * (m + 1), :].rearrange(
                        "(j p) d -> p j d", j=4
                    ),
                    in_=oo[:].rearrange("p (j d) -> p j d", j=4),
                )

    nc.compile()
    _prog_cache = nc
    return nc


def _prep_core_inputs(core, query, key, value):
    b, s = divmod(core, 2)
    qt = np.zeros((65, 2048), np.float32)
    qt[64, :] = 1.0
    kts = np.zeros((65, 4096), np.float32)
    vs = np.zeros((128, 2080), np.float32)
    ktm = np.ascontiguousarray(key[b, :3072, :].T)
    vaug = np.ones((S, 65), np.float32)
    vaug[:, :64] = value[b]
    vm = np.ascontiguousarray(
        vaug[: 24 * 128].reshape(24, 128, 65).transpose(1, 0, 2).reshape(128, 24 * 65)
    )
    for m in range(4):
        C = SLOT_C[m]
        c = 2 * m + 1 if s == 0 else 2 * m
        n = 4 * (c + 1)  # genuine k-tiles of this chunk
        qt[:64, 512 * m : 512 * (m + 1)] = query[b, 512 * c : 512 * (c + 1), :].T
        for p in range(8):
            col = slice(1024 * m + 128 * p, 1024 * m + 128 * (p + 1))
            vcol = slice(520 * m + 65 * p, 520 * m + 65 * (p + 1))
            if s == 0:
                t = C - 8 + p
            elif p < 4:
                kts[64, col] = NEG
                continue
            else:
                t = n - 8 + p  # p=4..7 -> diag tiles n-4..n-1
            kts[:64, col] = key[b, 128 * t : 128 * (t + 1), :].T
            vs[:, vcol] = vaug[128 * t : 128 * (t + 1), :]
    return {"qt": qt, "ktm": ktm, "kts": kts, "vm": vm, "vs": vs}


def run(query, key, value, trace=False):
    nc = _build_program()
    in_maps = [_prep_core_inputs(c, query, key, value) for c in range(NCORES)]
    res = run_bass_kernel_spmd(nc, in_maps, list(range(NCORES)), trace=trace)
    out = np.zeros((B, S, D), np.float32)
    for core in range(NCORES):
        b, s = divmod(core, 2)
        o = res.results[core]["o"]
        for m in range(4):
            c = 2 * m + 1 if s == 0 else 2 * m
            out[b, 512 * c : 512 * (c + 1), :] = o[512 * m : 512 * (m + 1), :]
    return out, res


def kernel(query, key, value):
    query = np.ascontiguousarray(np.asarray(query, dtype=np.float32))
    key = np.ascontiguousarray(np.asarray(key, dtype=np.float32))
    value = np.ascontiguousarray(np.asarray(value, dtype=np.float32))
    out, _ = run(query, key, value)
    return out
